# revision 1
# baseline (speedup 1.0000x reference)
"""Bass/Tile kernel for DeformableConv2d (offset conv + deform conv v1).

Per-core (data-parallel over batch, 1 image per NeuronCore):
  X:     [C=64, H, W] f32
  WOFF (host-prepped lhsT): [9, 64, 18]
  BOFF:  [18, 1]
  WCONV (host-prepped tap-pair lhsT): [5, 128, 64]
  OUT:   [64, H*W] f32

Pipeline:
  A. Zero-pad x into SBUF (PAD ring 16) -> x_pad [64, 160*160]; build a
     4-neighbor dup table in DRAM (row g = x_pad[:, g + (0,1,Wp,Wp+1)],
     256 f32 = 1KB rows) via PE transposes; offset conv on PE (9 shifted
     matmuls accumulated in PSUM, bias folded into the ACT evacuation);
     PE-transpose offsets to [wo, ho*18+ch].
  B. DVE: py/px -> exact floor (round +2^23 then is_lt fixup), fractions,
     clamped int16 table indices, 4 bilinear weights; DMA-rearrange indices
     into the 16-partition-wrapped layout dma_gather wants, replicate x8.
  C. Per (tap, ho-block): dma_gather 1024 rows of 1KB; DVE broadcast-mult by
     bilinear weights; 4 accumulating PE transposes fold the neighbor sum and
     the [sample, ch] -> [ch, sample] transpose into PSUM; evacuate to valT
     (tap-pair partition packing); einsum over (c, tap) on PE; DMA out.

Out-of-image samples land in the zero ring (indices clamped into it), which
reproduces the reference's per-neighbor zero-padding semantics exactly.
"""

from contextlib import ExitStack

import numpy as np

import concourse.bass as bass
import concourse.bacc as bacc
import concourse.mybir as mybir
from concourse.tile import TileContext
from concourse.masks import make_identity

F32 = mybir.dt.float32
I16 = mybir.dt.int16
AL = mybir.AluOpType
MAGIC = 12582912.0  # 1.5*2^23: f32 add rounds to nearest integer (symmetric range)


def build(H=128, W=128, C=64, O=64, PADR=16, dtype_tab=F32, num_devices=8):
    K = 3
    HW = H * W
    Hp, Wp = H + 2 * PADR, W + 2 * PADR
    HWp = Hp * Wp
    XPAD_LEN = HWp + 2 * Wp  # slack so the d=Wp+1 table slab can over-read
    NT = K * K  # 9 taps
    BLK = 1024 // W  # ho rows per block (1024 samples per gather)
    NBLK = H // BLK
    GPB = BLK * W // 128  # 128-sample groups per gather = 8
    NIDX = 128 * GPB  # 1024
    IDXC = NIDX // 16  # idx cols per gather = 64
    TPC = HW // 16  # wrapped idx cols per tap = 1024

    nc = bacc.Bacc("TRN2", num_devices=num_devices)
    X = nc.dram_tensor("X", [C, H, W], F32, kind="ExternalInput")
    WOFF = nc.dram_tensor("WOFF", [C, NT * 2 * NT], F32, kind="ExternalInput")
    BOFF = nc.dram_tensor("BOFF", [2 * NT, 1], F32, kind="ExternalInput")
    WCONV = nc.dram_tensor("WCONV", [128, 5 * O], F32, kind="ExternalInput")
    OUT = nc.dram_tensor("OUT", [O, HW], F32, kind="ExternalOutput")
    TAB = nc.dram_tensor("TAB", [HWp, 4 * C], dtype_tab, kind="Internal")

    with TileContext(nc) as tc, ExitStack() as ctx:
        const = ctx.enter_context(tc.tile_pool(name="const", bufs=1))
        ident = const.tile([128, 128], F32)
        make_identity(nc, ident[:])
        offT = const.tile([128, 18 * H], F32)  # [wo, ho*18+ch]

        # ---------------- stage A ----------------
        with (
            tc.tile_pool(name="stagea", bufs=1) as sa,
            tc.tile_pool(name="pa_tab", bufs=2, space="PSUM") as pa_tab,
            tc.tile_pool(name="pa_cv", bufs=2, space="PSUM") as pa_cv,
            tc.tile_pool(name="pa_ot", bufs=2, space="PSUM") as pa_ot,
            tc.tile_pool(name="eva", bufs=3) as eva,
        ):
            x_pad = sa.tile([C, XPAD_LEN], F32)
            nc.vector.memset(x_pad[:], 0.0)
            dst = x_pad[:, PADR * Wp + PADR : PADR * Wp + PADR + H * Wp]
            dst = dst.rearrange("c (h w) -> c h w", w=Wp)[:, :, :W]
            nc.sync.dma_start(dst, X[:, :, :])

            # table: HWp/128 chunks; 4 PE transposes + 1 ACT copy + 1 DMA each
            for gc in range(HWp // 128):
                g0 = gc * 128
                pt = pa_tab.tile([128, 4 * C], F32, space="PSUM", tag="tab")
                for n, d in enumerate((0, 1, Wp, Wp + 1)):
                    nc.tensor.transpose(
                        pt[:, C * n : C * (n + 1)],
                        x_pad[:, g0 + d : g0 + d + 128],
                        ident[:C, :C],
                    )
                ev = eva.tile([128, 4 * C], dtype_tab, tag="tabev")
                nc.scalar.copy(ev[:], pt[:])
                nc.sync.dma_start(TAB[g0 : g0 + 128, :], ev[:])

            # offset conv
            woff_sb = sa.tile([C, NT * 2 * NT], F32)
            nc.sync.dma_start(woff_sb[:], WOFF[:, :])
            boff_sb = sa.tile([2 * NT, 1], F32)
            nc.sync.dma_start(boff_sb[:], BOFF[:, :])
            CH = 512
            HCH = CH // W  # ho rows per conv chunk
            for pc in range(HW // CH):
                po = pa_cv.tile([2 * NT, CH], F32, space="PSUM", tag="oconv")
                base = (HCH * pc + PADR - 1) * Wp + PADR - 1
                for t in range(NT):
                    d = (t // 3) * Wp + (t % 3)
                    rhs = x_pad[:, base + d : base + d + HCH * Wp]
                    rhs = rhs.rearrange("c (h w) -> c h w", w=Wp)[:, :, :W]
                    nc.tensor.matmul(
                        po[:],
                        lhsT=woff_sb[:, 2 * NT * t : 2 * NT * (t + 1)],
                        rhs=rhs,
                        start=(t == 0),
                        stop=(t == NT - 1),
                    )
                offc = sa.tile([2 * NT, CH], F32, tag="offc")
                nc.scalar.activation(
                    offc[:],
                    po[:],
                    mybir.ActivationFunctionType.Identity,
                    bias=boff_sb[:],
                )
                # transpose this chunk's ho rows into offT right away
                ot = pa_ot.tile([128, HCH * 18], F32, space="PSUM", tag="offT")
                for s2 in range(HCH):
                    nc.tensor.transpose(
                        ot[:, 18 * s2 : 18 * (s2 + 1)],
                        offc[:, W * s2 : W * (s2 + 1)],
                        ident[: 2 * NT, : 2 * NT],
                    )
                nc.scalar.copy(
                    offT[:, 18 * HCH * pc : 18 * HCH * (pc + 1)], ot[:]
                )

        # ---------------- stage B ----------------
        idx_rep = const.tile([128, NT * TPC], I16)
        w4 = const.tile([128, NT * H, 4], F32)
        with tc.tile_pool(name="stageb", bufs=1) as sbp:
            ybases = []
            for g in range(3):
                yb = sbp.tile([128, H], F32, tag=f"ybase{g}")
                nc.gpsimd.iota(
                    yb[:], pattern=[[1, H]], base=PADR - 1 + g,
                    channel_multiplier=0, allow_small_or_imprecise_dtypes=True,
                )
                ybases.append(yb)
            xbase = sbp.tile([128, 3], F32)
            nc.gpsimd.iota(
                xbase[:], pattern=[[1, 3]], base=PADR - 1,
                channel_multiplier=1, allow_small_or_imprecise_dtypes=True,
            )
            idx_nat = sbp.tile([128, NT, H], I16)
            shp = [128, 3, H]
            for g in range(3):  # ky groups; taps t = 3g + kx
                p0 = offT[:].ap[0]
                dy = bass.AP(offT[:].tensor, offT[:].offset + 6 * g,
                             [p0, [2, 3], [18, H]])
                dx = bass.AP(offT[:].tensor, offT[:].offset + 6 * g + 1,
                             [p0, [2, 3], [18, H]])
                pys = sbp.tile(shp, F32, tag="pys")
                pxs = sbp.tile(shp, F32, tag="pxs")
                nc.vector.tensor_tensor(
                    pys[:], dy, ybases[g][:].unsqueeze(1).to_broadcast(shp), op=AL.add
                )
                nc.vector.tensor_tensor(
                    pxs[:], dx, xbase[:].unsqueeze(2).to_broadcast(shp), op=AL.add
                )

                def floorfrac(p, sfx):
                    r = sbp.tile(shp, F32, tag="rnd" + sfx)
                    nc.vector.tensor_scalar(r[:], p[:], MAGIC, -MAGIC,
                                            op0=AL.add, op1=AL.add)
                    d_ = sbp.tile(shp, F32, tag="dlt" + sfx)
                    nc.vector.tensor_tensor(d_[:], p[:], r[:], op=AL.subtract)
                    ng = sbp.tile(shp, F32, tag="ngt" + sfx)
                    nc.vector.tensor_scalar(ng[:], d_[:], 0.0, None, op0=AL.is_lt)
                    fl = sbp.tile(shp, F32, tag="flr" + sfx)
                    nc.vector.tensor_tensor(fl[:], r[:], ng[:], op=AL.subtract)
                    fr = sbp.tile(shp, F32, tag="frc" + sfx)
                    nc.vector.tensor_tensor(fr[:], d_[:], ng[:], op=AL.add)
                    return fl, fr

                y0, ly = floorfrac(pys, "y")
                x0, lx = floorfrac(pxs, "x")
                nc.vector.tensor_scalar(y0[:], y0[:], float(Hp - 2), 0.0,
                                        op0=AL.min, op1=AL.max)
                nc.vector.tensor_scalar(x0[:], x0[:], float(Wp - 2), 0.0,
                                        op0=AL.min, op1=AL.max)
                gf = sbp.tile(shp, F32, tag="gf")
                nc.vector.scalar_tensor_tensor(
                    gf[:], in0=y0[:], scalar=float(Wp), in1=x0[:],
                    op0=AL.mult, op1=AL.add,
                )
                nc.vector.tensor_copy(idx_nat[:, 3 * g : 3 * (g + 1), :], gf[:])
                omy = sbp.tile(shp, F32, tag="omy")
                omx = sbp.tile(shp, F32, tag="omx")
                nc.vector.tensor_scalar(omy[:], ly[:], -1.0, 1.0,
                                        op0=AL.mult, op1=AL.add)
                nc.vector.tensor_scalar(omx[:], lx[:], -1.0, 1.0,
                                        op0=AL.mult, op1=AL.add)
                wp0 = w4[:].ap[0]
                for n, (wy, wx) in enumerate(
                    ((omy, omx), (omy, lx), (ly, omx), (ly, lx))
                ):
                    wdst = bass.AP(
                        w4[:].tensor, w4[:].offset + 4 * H * 3 * g + n,
                        [wp0, [4 * H, 3], [4, H]],
                    )
                    nc.vector.tensor_tensor(wdst, wy[:], wx[:], op=AL.mult)

            # wrapped rearrange: [wo=(w16+16*w8), t, ho] -> [w16, t*TPC+ho*8+w8]
            idx_w = sbp.tile([16, NT * TPC], I16)
            for w8 in range(8):
                src = idx_nat[16 * w8 : 16 * (w8 + 1), :, :]
                dstw = bass.AP(
                    idx_w[:].tensor, idx_w[:].offset + w8,
                    [idx_w[:].ap[0], [TPC, NT], [8, H]],
                )
                nc.sync.dma_start(dstw, src)
            for r in range(8):
                nc.sync.dma_start(idx_rep[16 * r : 16 * (r + 1), :], idx_w[:])

        # ---------------- stage C ----------------
        with (
            tc.tile_pool(name="wc", bufs=1) as wcp,
            tc.tile_pool(name="gat", bufs=3) as gat,
            tc.tile_pool(name="mval", bufs=3) as mval,
            tc.tile_pool(name="vt", bufs=2) as vtp,
            tc.tile_pool(name="pc_v", bufs=4, space="PSUM") as pc_v,
            tc.tile_pool(name="pc_o", bufs=2, space="PSUM") as pc_o,
            tc.tile_pool(name="oev", bufs=3) as oev,
        ):
            wconv_sb = wcp.tile([128, 5 * O], F32)
            nc.sync.dma_start(wconv_sb[:], WCONV[:, :])
            for blk in range(NBLK):
                valT = vtp.tile([128, 5 * BLK * W], F32, tag="valT")
                for t in range(NT):
                    gt = gat.tile([128, GPB, 4 * C], dtype_tab, tag="gt")
                    icol = t * TPC + blk * IDXC
                    nc.gpsimd.dma_gather(
                        out_ap=gt[:],
                        in_ap=TAB[:],
                        idxs_ap=idx_rep[:, icol : icol + IDXC],
                        num_idxs=NIDX,
                        num_idxs_reg=NIDX,
                        elem_size=4 * C,
                    )
                    m = mval.tile([128, GPB, 4, C], F32, tag="m")
                    wsl = w4[:, t * H + blk * BLK : t * H + (blk + 1) * BLK, :]
                    nc.vector.tensor_tensor(
                        m[:],
                        gt[:].rearrange("p s (n c) -> p s n c", n=4),
                        wsl.unsqueeze(3).to_broadcast([128, BLK, 4, C]),
                        op=AL.mult,
                    )
                    # neighbor-sum + transpose on PE; 4 sample-groups per PSUM
                    pbase = 64 * (t % 2)
                    cbase = (t // 2) * BLK * W
                    for half in range(GPB // 4):
                        pv = pc_v.tile([64, 512], F32, space="PSUM", tag="pv")
                        for s in range(4):
                            for n in range(4):
                                nc.tensor.matmul(
                                    pv[:, 128 * s : 128 * (s + 1)],
                                    lhsT=m[:, 4 * half + s, n, :],
                                    rhs=ident[:],
                                    is_transpose=True,
                                    start=(n == 0),
                                    stop=(n == 3),
                                )
                        dstv = valT[pbase : pbase + 64,
                                    cbase + 512 * half : cbase + 512 * (half + 1)]
                        if (t + half) % 2 == 0:
                            nc.scalar.copy(dstv, pv[:])
                        else:
                            nc.vector.tensor_copy(dstv, pv[:])
                # einsum over tap pairs
                for pc2 in range(BLK * W // 512):
                    ops = pc_o.tile([O, 512], F32, space="PSUM", tag="ops")
                    for kc in range(5):
                        kk = 128 if kc < 4 else 64
                        nc.tensor.matmul(
                            ops[:],
                            lhsT=wconv_sb[:kk, O * kc : O * (kc + 1)],
                            rhs=valT[:kk, kc * BLK * W + 512 * pc2 :
                                     kc * BLK * W + 512 * (pc2 + 1)],
                            start=(kc == 0),
                            stop=(kc == 4),
                        )
                    ev = oev.tile([O, 512], F32, tag="outev")
                    nc.scalar.copy(ev[:], ops[:])
                    nc.sync.dma_start(
                        OUT[:, blk * BLK * W + 512 * pc2 :
                            blk * BLK * W + 512 * (pc2 + 1)],
                        ev[:],
                    )
    nc.compile()
    return nc


def host_prep(x, w_off, b_off, w_conv):
    """Full inputs -> list of per-core in_maps (one image per core)."""
    B, C, H, W = x.shape
    NT = 9
    O = w_conv.shape[0]
    # woff_sb[c, t*18+ch] = w_off[ch, c, ky, kx], t = ky*3+kx
    woff_l = np.ascontiguousarray(
        np.transpose(w_off, (1, 2, 3, 0)).reshape(C, NT * 2 * NT)
    ).astype(np.float32)
    wt = np.transpose(w_conv.reshape(O, C, NT), (2, 1, 0))  # [t, c, o]
    wpair = np.zeros((5, 128, O), np.float32)
    for kc in range(4):
        wpair[kc, :C] = wt[2 * kc]
        wpair[kc, C:2*C] = wt[2 * kc + 1]
    wpair[4, :C] = wt[8]
    # wconv_sb[c2, kc*O+o] = wpair[kc, c2, o]
    wconv_l = np.ascontiguousarray(
        np.transpose(wpair, (1, 0, 2)).reshape(128, 5 * O)
    ).astype(np.float32)
    boff = b_off.reshape(2 * NT, 1).astype(np.float32)
    return [
        {"X": np.ascontiguousarray(x[b]).astype(np.float32),
         "WOFF": woff_l, "BOFF": boff, "WCONV": wconv_l}
        for b in range(B)
    ]


_NC_CACHE = {}


def _get_nc():
    if "nc" not in _NC_CACHE:
        _NC_CACHE["nc"] = build(H=128, W=128, num_devices=8)
    return _NC_CACHE["nc"]


def kernel(x, w_off, b_off, w_conv):
    """Full-input entry point: shards batch across 8 NeuronCores."""
    from concourse.bass_utils import run_bass_kernel_spmd

    x = np.asarray(x, dtype=np.float32)
    w_off = np.asarray(w_off, dtype=np.float32)
    b_off = np.asarray(b_off, dtype=np.float32)
    w_conv = np.asarray(w_conv, dtype=np.float32)
    B, C, H, W = x.shape
    nc = _get_nc()
    in_maps = host_prep(x, w_off, b_off, w_conv)
    res = run_bass_kernel_spmd(nc, in_maps, core_ids=list(range(B)))
    out = np.stack([res.results[b]["OUT"].reshape(64, H, W) for b in range(B)])
    return out.astype(np.float32)

